# revision 17
# baseline (speedup 1.0000x reference)
"""CARAFE forward on 8 Trainium2 NeuronCores, data-parallel over batch.

Per core (1 sample):
  1. x loaded dense, replicate-padded to 66x66 (f32), then cast+duplicated to
     xpad bf16 [128=(g,chi), 4cc, 66, 66]: partition halves g=0/1 hold the
     SAME 64-channel chunk (duplication lets one DVE multiply serve two
     pixel-shuffle quadrants at once -> kernel-map broadcast volume halves).
  2. 1x1 conv compressor (PE, 4x64-channel contraction chunks); BN batch
     sums/sumsqs accumulated during PSUM evacuation via accum_out;
     AllReduce over 8 cores (exact sync-BN); BN+ReLU applied in place.
  3. 3x3 encoder conv (PE, 9 taps PSUM-accumulated), fused bias+exp on
     evacuation; softmax over H (faithful to source nn.Softmax(dim=1)).
  4. reassembly per 16-row block qq, quadrant-pair sig, channel chunk cc:
     kernel maps broadcast SBUF->SBUF to 64-replicated mexp tiles with the
     tap's kj-shift folded into the broadcast source offset (66-wide rows),
     so a single unshifted xpad serves all 9 taps with 4B-aligned DVE 2x
     reads; products (DVE + gpsimd) accumulate over taps on the PE via
     identity matmuls; PSUM quadrant halves merged by Act copies into an
     olin staging tile whose out-DMA is 16KB-contiguous per channel.
"""
import numpy as np

import concourse.bass as bass
import concourse.tile as tile
from concourse import bacc, mybir
from concourse.bass_utils import run_bass_kernel_spmd
from concourse.masks import make_identity

F32 = mybir.dt.float32
BF16 = mybir.dt.float16  # 16-bit compute dtype (fp16: 11-bit mantissa)
AX = mybir.AxisListType
OP = mybir.AluOpType
AF = mybir.ActivationFunctionType

B, C, H, W = 8, 256, 64, 64
CC = 64          # compressed channels
S = 2            # scale factor
K = 3            # kernel size
E = S * S * K * K  # 36 encoder channels
EPS = 1e-5
NCORES = 8
WP = W + 2       # padded width (66); rows also 66
NPIX = H * W
KROW = H * WP + 2  # kern_pad row length: 64 rows at 66-pitch + guard elems
NQQ = 4          # row blocks
QH = H // NQQ    # 16 h-rows per block
NDVE_T = 7       # taps 0..6 on DVE, 7..8 on gpsimd (gpsimd TT is ~4x slower)


def _ap(t, ap, extra_offset=0):
    return bass.AP(tensor=t.tensor, offset=t.offset + extra_offset, ap=ap)


def build():
    nc = bacc.Bacc("TRN2", target_bir_lowering=False, debug=False,
                   num_devices=NCORES)
    x_d = nc.dram_tensor("x", [C, H, W], F32, kind="ExternalInput").ap()
    w1_d = nc.dram_tensor("w1", [CC, C], F32, kind="ExternalInput").ap()
    b1_d = nc.dram_tensor("b1", [CC, 1], F32, kind="ExternalInput").ap()
    gamma_d = nc.dram_tensor("gamma", [CC, 1], F32, kind="ExternalInput").ap()
    beta_d = nc.dram_tensor("beta", [CC, 1], F32, kind="ExternalInput").ap()
    w2_d = nc.dram_tensor("w2", [E, CC * K * K], F32, kind="ExternalInput").ap()
    b2_d = nc.dram_tensor("b2", [E, 1], F32, kind="ExternalInput").ap()
    out_d = nc.dram_tensor("out", [C, S * H, S * W], F32, kind="ExternalOutput").ap()

    with tile.TileContext(nc) as tc:
        with (
            tc.tile_pool(name="persist", bufs=1) as persist,
            tc.tile_pool(name="small", bufs=1) as small,
            tc.tile_pool(name="dram", bufs=1, space="DRAM") as dram,
        ):
            # ---------- constants & weights ----------
            ident = persist.tile([128, 128], F32)
            make_identity(nc, ident)
            ident_bf = persist.tile([128, 128], BF16)
            nc.scalar.copy(out=ident_bf, in_=ident)

            w1_sb = small.tile([CC, C], F32)
            nc.sync.dma_start(out=w1_sb, in_=w1_d)
            w2_sb = small.tile([E, CC * K * K], F32)
            nc.sync.dma_start(out=w2_sb, in_=w2_d)
            b1_sb = small.tile([CC, 1], F32)
            nc.sync.dma_start(out=b1_sb, in_=b1_d)
            gamma_sb = small.tile([CC, 1], F32)
            nc.sync.dma_start(out=gamma_sb, in_=gamma_d)
            beta_sb = small.tile([CC, 1], F32)
            nc.sync.dma_start(out=beta_sb, in_=beta_d)
            b2_sb = small.tile([E, 1], F32)
            nc.sync.dma_start(out=b2_sb, in_=b2_d)

            # transposed weights via PE (stored bf16)
            w1T = persist.tile([64, 4, CC], BF16)     # (c in chunk, cc, o)
            w2T = persist.tile([CC, K * K, E], BF16)  # (c, tap, e)
            with tc.tile_pool(name="tp", bufs=2, space="PSUM") as tps:
                for kap in range(4):
                    pt = tps.tile([64, CC], F32, tag="w1t")
                    # columns c = 4j + kap, matching the xpad channel gather
                    w1v = _ap(w1_sb[:, :], [w1_sb[:, :].ap[0], [4, 64]],
                              extra_offset=kap)
                    nc.tensor.transpose(pt, w1v, ident[:CC, :CC])
                    nc.scalar.copy(out=w1T[:, kap, :], in_=pt)
                for t in range(K * K):
                    pt2 = tps.tile([CC, E], F32, tag="w2t")
                    # w2_sb row e holds (c, tap) flat; view tap t: (E, CC) stride K*K
                    src = _ap(w2_sb[:, :], [w2_sb[:, :].ap[0], [K * K, CC]],
                              extra_offset=t)
                    nc.tensor.transpose(pt2, src, ident[:E, :E])
                    nc.scalar.copy(out=w2T[:, t, :], in_=pt2)

            # ---------- x: kap-gathered load, replicate-pad, cast+dup ----------
            # The reference's reshape scramble sends input channel c to output
            # channel s2*64 + c//4 at quadrant kap = c%4. Absorb it here:
            # xpad[p=(g*64+j), kap, a, b] = x[4j+kap, clamp(a-1), clamp(b-1)],
            # duplicated across partition halves g (so one DVE product serves
            # two s2 kernel maps at once).
            xpad = persist.tile([128, 4, WP, WP], BF16)
            with tc.tile_pool(name="xfp", bufs=2) as xfp:
                for kap in range(4):
                    stag = xfp.tile([128, WP, WP], F32, tag="stag")
                    for g in range(2):
                        src = _ap(x_d[0:1, 0:1, 0:1],
                                  [[4 * NPIX, 64], [1, NPIX]],
                                  extra_offset=kap * NPIX)
                        nc.sync.dma_start(
                            out=stag[g * 64:g * 64 + 64, 1:H + 1, 1:W + 1],
                            in_=src)
                    nc.vector.tensor_copy(out=stag[:, 1:H + 1, 0:1],
                                          in_=stag[:, 1:H + 1, 1:2])
                    nc.vector.tensor_copy(out=stag[:, 1:H + 1, WP - 1:WP],
                                          in_=stag[:, 1:H + 1, WP - 2:WP - 1])
                    nc.vector.tensor_copy(out=stag[:, 0:1, :],
                                          in_=stag[:, 1:2, :])
                    nc.vector.tensor_copy(out=stag[:, WP - 1:WP, :],
                                          in_=stag[:, WP - 2:WP - 1, :])
                    cast_eng = [nc.scalar.copy, nc.vector.tensor_copy][kap % 2]
                    cast_eng(out=xpad[:, kap, :, :], in_=stag)

            # ---------- compressor + BN stats (fused into evacuation) ----------
            comp = persist.tile([CC, WP, WP], BF16)
            nc.vector.memset(comp[:, 0:1, :], 0.0)
            nc.vector.memset(comp[:, WP - 1:WP, :], 0.0)
            nc.vector.memset(comp[:, :, 0:1], 0.0)
            nc.vector.memset(comp[:, :, WP - 1:WP], 0.0)
            NCH = 8  # h rows per chunk
            ssum = small.tile([CC, 8], F32)
            ssq = small.tile([CC, 8], F32)
            with (
                tc.tile_pool(name="cps", bufs=2, space="PSUM") as cps,
                tc.tile_pool(name="dmp", bufs=2) as dmp,
            ):
                for hc in range(8):
                    pc = cps.tile([CC, NCH, W], F32, tag="comp")
                    for kap in range(4):
                        nc.tensor.matmul(
                            pc, w1T[:, kap, :],
                            xpad[0:64, kap, 1 + hc * NCH:1 + (hc + 1) * NCH,
                                 1:W + 1],
                            start=(kap == 0), stop=(kap == 3))
                    nc.scalar.activation(
                        out=comp[:, 1 + hc * NCH:1 + (hc + 1) * NCH, 1:W + 1],
                        in_=pc, func=AF.Identity, bias=b1_sb, scale=1.0,
                        accum_out=ssum[:, hc:hc + 1])
                    dump = dmp.tile([CC, NCH, W], F32)
                    nc.scalar.activation(
                        out=dump, in_=pc, func=AF.Square, bias=b1_sb, scale=1.0,
                        accum_out=ssq[:, hc:hc + 1])

            stats = small.tile([CC, 2], F32)
            nc.vector.tensor_reduce(out=stats[:, 0:1], in_=ssum, axis=AX.X,
                                    op=OP.add)
            nc.vector.tensor_reduce(out=stats[:, 1:2], in_=ssq, axis=AX.X,
                                    op=OP.add)

            # ---------- AllReduce (exact sync-BN) ----------
            cc_in = dram.tile([CC, 2], F32)
            cc_out = dram.tile([CC, 2], F32)
            nc.gpsimd.dma_start(out=cc_in[:], in_=stats)
            nc.gpsimd.collective_compute(
                "AllReduce", OP.add,
                replica_groups=[list(range(NCORES))],
                ins=[cc_in[:].opt()], outs=[cc_out[:].opt()])
            gstats = small.tile([CC, 2], F32)
            nc.gpsimd.dma_start(out=gstats, in_=cc_out[:])

            mu = small.tile([CC, 1], F32)
            var = small.tile([CC, 1], F32)
            scl = small.tile([CC, 1], F32)
            shf = small.tile([CC, 1], F32)
            inv_n = 1.0 / (B * NPIX)
            nc.vector.tensor_scalar_mul(out=mu, in0=gstats[:, 0:1], scalar1=inv_n)
            nc.vector.tensor_scalar_mul(out=var, in0=gstats[:, 1:2], scalar1=inv_n)
            nc.vector.tensor_tensor(out=shf, in0=mu, in1=mu, op=OP.mult)
            nc.vector.tensor_tensor(out=var, in0=var, in1=shf, op=OP.subtract)
            # scl = gamma / sqrt(var + eps); shf = beta - mu * scl
            eps_sb = small.tile([CC, 1], F32)
            nc.vector.memset(eps_sb, EPS)
            nc.scalar.activation(out=var, in_=var, func=AF.Sqrt, bias=eps_sb,
                                 scale=1.0)
            nc.vector.reciprocal(out=var, in_=var)
            nc.vector.tensor_tensor(out=scl, in0=gamma_sb, in1=var, op=OP.mult)
            nc.vector.tensor_tensor(out=shf, in0=mu, in1=scl, op=OP.mult)
            nc.vector.tensor_tensor(out=shf, in0=beta_sb, in1=shf, op=OP.subtract)
            interior = comp[:, 1:H + 1, 1:W + 1]
            nc.scalar.activation(out=interior, in_=interior, func=AF.Relu,
                                 bias=shf, scale=scl)

            # ---------- encoder conv + fused exp ----------
            # kern_pad[ch, 1 + h*66 + u] = kern[ch, h, u-1]; 66-pitch rows so
            # a tap's (rows, cols) broadcast source merges into one contiguous
            # dim, with the kj-shift a pure scalar offset.
            kern_pad = persist.tile([E, KROW], BF16)
            nc.vector.memset(kern_pad, 0.0)
            with (
                tc.tile_pool(name="enc", bufs=1) as encp,
                tc.tile_pool(name="eps", bufs=2, space="PSUM") as eps_pool,
            ):
                eexp = encp.tile([E, H, W], F32)
                for hc in range(8):
                    pe = eps_pool.tile([E, NCH, W], F32, tag="enc")
                    for t in range(K * K):
                        ki, kj = t // K, t % K
                        nc.tensor.matmul(
                            pe, w2T[:, t, :],
                            comp[:, hc * NCH + ki:hc * NCH + ki + NCH,
                                 kj:kj + W],
                            start=(t == 0), stop=(t == K * K - 1))
                    nc.scalar.activation(
                        out=eexp[:, hc * NCH:(hc + 1) * NCH, :], in_=pe,
                        func=AF.Exp, bias=b2_sb, scale=1.0)

                # softmax over h (axis=1 of (b, h, w, s2, k2))
                zrec = small.tile([E, W], F32)
                ee = eexp[:, :, :]
                ee_wh = _ap(ee, [ee.ap[0], [1, W], [W, H]])
                nc.vector.tensor_reduce(out=zrec, in_=ee_wh, axis=AX.X, op=OP.add)
                nc.vector.reciprocal(out=zrec, in_=zrec)
                zb = zrec[:, :]
                kview = _ap(kern_pad[:, :], [kern_pad[:, :].ap[0],
                                             [WP, H], [1, W]],
                            extra_offset=2)
                nc.vector.tensor_tensor(
                    out=kview, in0=ee,
                    in1=_ap(zb, [zb.ap[0], [0, H], [1, W]]),
                    op=OP.mult)
            # DRAM copy of kern_pad: broadcast source (HWDGE issues a
            # DRAM-sourced partition-broadcast ~7x faster than an SBUF one)
            kern_dr = dram.tile([E, KROW], BF16)
            nc.gpsimd.dma_start(out=kern_dr[:], in_=kern_pad)

            # ---------- reassembly ----------
            # out[s2*64+j, 2h+kap//2, (kap%2)*64+w]
            #   = sum_t xpad[p=(g,j), kap, h+ki, w+kj] * kern[s2*9+t, h, w]
            # with s2 = 2*sig+g; identity matmuls accumulate taps in PSUM,
            # psum half g evacuates to olin[s2] quadrant kap.
            with (
                tc.tile_pool(name="mexp", bufs=3) as mpool,
                tc.tile_pool(name="prod", bufs=10) as ppool,
                tc.tile_pool(name="olin", bufs=3) as opool,
                tc.tile_pool(name="ops", bufs=4, space="PSUM") as ops_pool,
            ):
                evac_engs = [nc.scalar.copy]
                n_evac = 0
                for qq in range(NQQ):
                    mexps = []
                    for sig in range(2):
                        mexp = mpool.tile([128, K * K, QH, WP], BF16,
                                          name=f"mexp_{qq}_{sig}", tag="mexp")
                        # per (g, kj): taps {kj, kj+3, kj+6} from kern_dr rows
                        # (2sig+g)*9 + kj + 3m, replicated to 64 partitions;
                        # the 66-pitch rows make (rows, cols) one contiguous
                        # dim and the kj-shift a scalar offset.
                        for g in range(2):
                            for kj in range(K):
                                ch0 = (2 * sig + g) * K * K + kj
                                src = _ap(kern_dr[0:1, 0:1],
                                          [[0, 64], [K * KROW, K],
                                           [1, QH * WP]],
                                          extra_offset=ch0 * KROW + 2 - kj
                                          + qq * QH * WP)
                                dst = _ap(mexp[g * 64:g * 64 + 64, 0, 0, 0:1],
                                          [mexp[g * 64:g * 64 + 64, 0, 0,
                                                0:1].ap[0],
                                           [K * QH * WP, K], [1, QH * WP]],
                                          extra_offset=kj * QH * WP)
                                eng = nc.sync if (g + kj) % 2 == 0 else nc.scalar
                                eng.dma_start(out=dst, in_=src)
                        mexps.append(mexp)

                    for sig in range(2):
                        olins = [opool.tile([64, 2 * QH, S * W], F32,
                                            name=f"olin_{qq}_{sig}_{g}",
                                            tag="olin")
                                 for g in range(2)]
                        for kap in range(4):
                            hb, wb = kap // 2, kap % 2
                            prods = []
                            for t in range(K * K):
                                ki = t // K
                                prod = ppool.tile([128, QH, WP], BF16)
                                eng = nc.vector if t < NDVE_T else nc.gpsimd
                                eng.tensor_tensor(
                                    out=prod,
                                    in0=xpad[:, kap, qq * QH + ki:
                                             qq * QH + ki + QH, :],
                                    in1=mexps[sig][:, t, :, :], op=OP.mult)
                                prods.append(prod)
                            pss = [ops_pool.tile([128, 512], F32, tag=f"o{i}",
                                                 name=f"ps_{qq}_{sig}_{kap}_{i}")
                                   for i in range(2)]
                            for t in range(K * K):
                                kj = t % K
                                for blk in range(2):
                                    rhs = prods[t][:, blk * 8:blk * 8 + 8,
                                                   kj:kj + W]
                                    nc.tensor.matmul(
                                        pss[blk], ident_bf, rhs,
                                        start=(t == 0), stop=(t == K * K - 1))
                            # psum half g (s2=2sig+g, channels j) -> olin[g]
                            # rows 2*(blk*8+hl)+hb, cols wb*64+w
                            for blk in range(2):
                                for g in range(2):
                                    ob = olins[g][:, :, :]
                                    dst = _ap(
                                        ob, [ob.ap[0], [2 * S * W, NCH], [1, W]],
                                        extra_offset=(2 * blk * NCH + hb)
                                        * S * W + wb * W)
                                    evac_engs[n_evac % len(evac_engs)](
                                        out=dst,
                                        in_=pss[blk][g * 64:g * 64 + 64, :])
                                    n_evac += 1
                        for g in range(2):
                            s2 = 2 * sig + g
                            nc.scalar.dma_start(
                                out=out_d[s2 * 64:s2 * 64 + 64,
                                          qq * 2 * QH:(qq + 1) * 2 * QH, :],
                                in_=olins[g])
    nc.compile()
    return nc


_NC_CACHE = None


def _get_nc():
    global _NC_CACHE
    if _NC_CACHE is None:
        _NC_CACHE = build()
    return _NC_CACHE


def _make_in_maps(inputs):
    x = np.ascontiguousarray(inputs["x"], dtype=np.float32)
    in_maps = []
    for b in range(NCORES):
        in_maps.append({
            "x": np.ascontiguousarray(x[b]),
            "w1": np.ascontiguousarray(inputs["w1"], dtype=np.float32),
            "b1": np.ascontiguousarray(np.asarray(inputs["b1"], dtype=np.float32).reshape(CC, 1)),
            "gamma": np.ascontiguousarray(np.asarray(inputs["gamma"], dtype=np.float32).reshape(CC, 1)),
            "beta": np.ascontiguousarray(np.asarray(inputs["beta"], dtype=np.float32).reshape(CC, 1)),
            "w2": np.ascontiguousarray(np.asarray(inputs["w2"], dtype=np.float32).reshape(E, CC * K * K)),
            "b2": np.ascontiguousarray(np.asarray(inputs["b2"], dtype=np.float32).reshape(E, 1)),
        })
    return in_maps


def kernel(x, w1, b1, gamma, beta, w2, b2, **kwargs):
    in_maps = _make_in_maps(dict(x=x, w1=w1, b1=b1, gamma=gamma, beta=beta,
                                 w2=w2, b2=b2))
    nc = _get_nc()
    res = run_bass_kernel_spmd(nc, in_maps, core_ids=list(range(NCORES)))
    return np.stack([res.results[b]["out"] for b in range(NCORES)], axis=0)
